# revision 1
# baseline (speedup 1.0000x reference)
"""Trainium2 Bass kernel for nn_CollectiveDecActorTaxi0Obs (gnn_message_passing).

Computes, for obs [32768, 48], per-zone dense heads W [81, 48, 5] (+bias b,
adjacency idx/mask [81, 5]):
    logits = einsum('bd,ndk->bnk', obs, W) + b ; masked softmax over k
    out[b, n, idx[n, k]] += probs[b, n, k]              -> [32768, 81, 81] f32

Strategy (pure data parallelism, 8 cores, batch-sharded 4096 rows each):
  All small operands (W, b, idx, mask) are folded on the host into constant
  matrices so the device only runs matmuls + exp + elementwise:
    - Wa [49, 448]:   W flattened to padded slot columns with a bias row
                      appended; masked slots get bias -1e9 (exp underflows to
                      exactly 0, matching the reference's where(mask>0,.,-1e9)).
    - ob_p [pw, 81]:  0/1 slot->zone map -> per-zone sums of exp (softmax den)
    - E [81, 448]:    expands per-zone reciprocal denom back to slot rows
    - S [128, 6561]:  0/1 selection matrix built from idx; the scatter into
                      the 81-wide adjacency vector IS a matmul probs @ S
                      (duplicate idx entries accumulate, like .at[].add).
  fp32 matmuls on TRN2 cost 2 weight passes x 2 cycles/col; bf16 costs 1 x 1.
  probs is split hi+lo into two bf16 tensors (x == hi + lo to ~2^-18 relative)
  that are STACKED on the contraction axis: since both multiply the same 0/1
  S matrix (exact in bf16), one K=128 bf16 matmul computes hi@S + lo@S at a
  quarter of the fp32 cost (matmul time scales with N only). The same split
  handles the recip-denominator expansion. The softmax denominator matmul
  stays fp32 for accuracy; its reciprocal runs on the vector engine.

  Slot layout: 14 scatter groups of 6 zones (30 slots; last group 3 zones),
  two groups -> one 64-row half-chunk [A|pad|B], two half-chunks -> one
  128-row pair for the fp32 logits/den stage. The split tiles pcat hold the
  half-chunk's hi rows at 0..63 and lo rows at 64..127, so every scatter
  matmul is a full-K (128) single pass whose unused rows hit zero S rows.

  Everything runs in a transposed layout (batch on the free dim) until the
  scatter matmul, whose PSUM output lands batch-on-partitions so dense
  [128, 6561] tiles stream to DRAM with unit-stride rows.
  The kernel is HBM-write-bound: 860 MB of output, ~107 MB/core, ~320 us
  at the ~358 GB/s per-core HBM limit.
"""

import os
import sys

sys.path.insert(0, "/opt/trn_rl_repo")

import numpy as np

NZ = 81          # zones
D = 48           # obs dim used
DA = D + 1       # + bias row
KADJ = 5         # adjacency slots per zone
NCORES = 8
BATCH = 32768
BLOC = BATCH // NCORES   # 4096 rows per core
BF = 512                 # batch free-dim block (matmul N limit for fp32 PSUM)
P = 128
NEG = np.float32(-1e9)

ZPG = 6                        # zones per scatter group (30 slots + 2 pad)
NGRP = 14                      # groups: 13x6 zones + 1x3 zones
GRP_NZ = [6] * 13 + [3]
GRP_COL = [486 * g for g in range(14)]          # output column offset
PW_PAIR = [128, 128, 128, 64]  # used rows per pair (pair 3 = one half-chunk)
PADW = 448                     # 3*128 + 64 packed columns

LAST_RESULTS = None


def _slot(n, k):
    """(zone, k) -> (pair, row_in_pair, halfchunk, row_in_halfchunk_hi)."""
    g = n // ZPG
    zz = n % ZPG
    hc = g // 2
    p = hc // 2
    row_hi = 32 * (g % 2) + KADJ * zz + k       # 0..61 within half-chunk
    row_pair = 64 * (hc % 2) + row_hi
    return p, row_pair, hc, row_hi


def _build_consts(W, b, idx, mask):
    import ml_dtypes

    bf = ml_dtypes.bfloat16
    W = np.asarray(W, np.float32)
    b = np.asarray(b, np.float32)
    idx = np.asarray(idx)
    mask = np.asarray(mask, np.float32)

    Wa = np.zeros((DA, PADW), np.float32)
    E = np.zeros((NZ, PADW), bf)
    ob = [np.zeros((PW_PAIR[p], NZ), np.float32) for p in range(4)]
    S = np.zeros((P, NZ * NZ), bf)

    for n in range(NZ):
        for k in range(KADJ):
            p, rp, hc, rh = _slot(n, k)
            col = 128 * p + rp
            if mask[n, k] > 0:
                Wa[:D, col] = W[n, :, k]
                Wa[D, col] = b[n, k]
            else:
                Wa[D, col] = NEG
            E[n, col] = 1.0
            ob[p][rp, n] = 1.0
            ocol = n * NZ + int(idx[n, k])
            S[rh, ocol] = 1.0        # hi rows
            S[64 + rh, ocol] = 1.0   # lo rows
    return Wa, E, ob, S


def _build_program(bloc):
    from concourse import bacc, mybir
    import concourse.tile as tile

    f32 = mybir.dt.float32
    bf16 = mybir.dt.bfloat16
    AF = mybir.ActivationFunctionType
    OP = mybir.AluOpType
    nc = bacc.Bacc("TRN2", target_bir_lowering=False, debug=False)

    xTa_d = nc.declare_dram_parameter("xTa", [DA, bloc], f32, isOutput=False)
    Wa_d = nc.declare_dram_parameter("Wa", [DA, PADW], f32, isOutput=False)
    E_d = nc.declare_dram_parameter("E", [NZ, PADW], bf16, isOutput=False)
    ob_d = [
        nc.declare_dram_parameter(f"ob{p}", [PW_PAIR[p], NZ], f32, isOutput=False)
        for p in range(4)
    ]
    S_d = nc.declare_dram_parameter("S", [P, NZ * NZ], bf16, isOutput=False)
    out_d = nc.declare_dram_parameter("out", [bloc, NZ * NZ], f32, isOutput=True)

    n_blk = bloc // BF
    n_sub = BF // P

    with tile.TileContext(nc) as tc:
        with (
            tc.tile_pool(name="const", bufs=1) as cpool,
            tc.tile_pool(name="work", bufs=2) as wpool,
            tc.tile_pool(name="outp", bufs=4) as opool,
            tc.tile_pool(name="ps_log", bufs=2, space="PSUM") as ps_log,
            tc.tile_pool(name="ps_den", bufs=1, space="PSUM") as ps_den,
            tc.tile_pool(name="ps_rf", bufs=2, space="PSUM") as ps_rf,
            tc.tile_pool(name="ps_sc", bufs=3, space="PSUM") as ps_sc,
        ):
            Wa_sb = cpool.tile([DA, PADW], f32, tag="Wa")
            nc.sync.dma_start(out=Wa_sb[:], in_=Wa_d[:])
            E_sb = cpool.tile([NZ, PADW], bf16, tag="E")
            nc.sync.dma_start(out=E_sb[:], in_=E_d[:])
            S_sb = cpool.tile([P, NZ * NZ], bf16, tag="S")
            nc.sync.dma_start(out=S_sb[:], in_=S_d[:])
            ob_sb = []
            for p in range(4):
                t = cpool.tile([PW_PAIR[p], NZ], f32, tag=f"ob{p}")
                nc.sync.dma_start(out=t[:], in_=ob_d[p][:])
                ob_sb.append(t)
            xTa_sb = cpool.tile([DA, bloc], f32, tag="xTa")
            nc.sync.dma_start(out=xTa_sb[:], in_=xTa_d[:])

            def emit_scatter(bs, pcat):
                for i in range(n_sub):
                    osb = opool.tile([P, NZ * NZ], f32, tag="osb")
                    for g in range(NGRP):
                        ncols = GRP_NZ[g] * NZ
                        colg = GRP_COL[g]
                        sc = ps_sc.tile([P, BF], f32, tag="scps")
                        nc.tensor.matmul(
                            sc[:, :ncols],
                            pcat[g // 2][:, i * P:(i + 1) * P],
                            S_sb[:, colg:colg + ncols],
                            start=True,
                            stop=True,
                        )
                        dst = osb[:, colg:colg + ncols]
                        if g % 5 < 3:
                            nc.scalar.copy(dst, sc[:, :ncols])
                        else:
                            nc.vector.tensor_copy(dst, sc[:, :ncols])
                    nc.sync.dma_start(
                        out=out_d[bs + i * P: bs + (i + 1) * P, :], in_=osb[:]
                    )

            prev = None
            for blk in range(n_blk):
                bs = blk * BF
                exT = []
                for p in range(4):
                    pw = PW_PAIR[p]
                    lg = ps_log.tile([P, BF], f32, tag="lg")
                    nc.tensor.matmul(
                        lg[:pw, :],
                        Wa_sb[:, 128 * p:128 * p + pw],
                        xTa_sb[:, bs:bs + BF],
                        start=True,
                        stop=True,
                    )
                    ex = wpool.tile([P, BF], f32, tag=f"exp{p}")
                    nc.scalar.activation(ex[:pw, :], lg[:pw, :], AF.Exp)
                    exT.append(ex)
                den_ps = ps_den.tile([NZ, BF], f32, tag="den")
                for p in range(4):
                    nc.tensor.matmul(
                        den_ps[:, :], ob_sb[p][:], exT[p][:PW_PAIR[p], :],
                        start=(p == 0), stop=(p == 3),
                    )
                rc = wpool.tile([NZ, BF], f32, tag="recipC")
                nc.vector.reciprocal(rc[:], den_ps[:])
                rhi = wpool.tile([NZ, BF], bf16, tag="rhi")
                nc.scalar.copy(rhi[:], rc[:])
                rlo = wpool.tile([NZ, BF], bf16, tag="rlo")
                nc.vector.tensor_tensor(out=rlo[:], in0=rc[:], in1=rhi[:], op=OP.subtract)
                pcat = []
                for p in range(4):
                    pw = PW_PAIR[p]
                    rf = ps_rf.tile([P, BF], f32, tag="rf")
                    nc.tensor.matmul(
                        rf[:pw, :], E_sb[:, 128 * p:128 * p + pw], rhi[:],
                        start=True, stop=False,
                    )
                    nc.tensor.matmul(
                        rf[:pw, :], E_sb[:, 128 * p:128 * p + pw], rlo[:],
                        start=False, stop=True,
                    )
                    for h in range(2 if pw == 128 else 1):
                        sl = slice(64 * h, 64 * h + 64)
                        pt = wpool.tile([64, BF], f32, tag=f"pt{2 * p + h}")
                        nc.vector.tensor_tensor(
                            out=pt[:, :], in0=exT[p][sl, :], in1=rf[sl, :], op=OP.mult
                        )
                        pc = wpool.tile([P, BF], bf16, tag=f"pcat{2 * p + h}")
                        nc.scalar.copy(pc[:64, :], pt[:, :])
                        nc.vector.tensor_tensor(
                            out=pc[64:, :],
                            in0=pt[:, :],
                            in1=pc[:64, :],
                            op=OP.subtract,
                        )
                        pcat.append(pc)
                if prev is not None:
                    emit_scatter(*prev)
                prev = (bs, pcat)
            emit_scatter(*prev)
    nc.compile()
    return nc


def _install_ntff_hook():
    """Shim antenv.axon_hooks (absent in this image) so trace=True can drive
    NRT profiling through libaxon_pjrt.so. Only used for self-profiling."""
    import types

    try:
        import antenv

        try:
            from antenv.axon_hooks import get_axon_ntff_profile_hook  # noqa: F401

            return True
        except ImportError:
            pass
        if "/root/.axon_site" not in sys.path:
            sys.path.insert(0, "/root/.axon_site")
        from trn_agent_boot.trn_boot import _ntff_profile_via_ctypes

        hook = _ntff_profile_via_ctypes("/opt/axon/libaxon_pjrt.so")
        mod = types.ModuleType("antenv.axon_hooks")
        state = {"hook": hook}
        mod.get_axon_ntff_profile_hook = lambda: state["hook"]
        mod.set_axon_ntff_profile_hook = lambda h: state.update(hook=h)
        sys.modules["antenv.axon_hooks"] = mod
        antenv.axon_hooks = mod
        return hook is not None
    except Exception as e:  # profiling is best-effort; never break the run
        print("ntff hook install failed:", e)
        return False


def kernel(obs, W, b, idx, mask):
    from concourse.bass_utils import run_bass_kernel_spmd

    global LAST_RESULTS
    trace = bool(int(os.environ.get("KBT_TRACE", "0")))
    if trace:
        trace = _install_ntff_hook()
    obs = np.asarray(obs, np.float32)
    Wa, E, ob, S = _build_consts(W, b, idx, mask)

    nc = _build_program(BLOC)

    consts = {"Wa": Wa, "E": E, "S": S}
    for p in range(4):
        consts[f"ob{p}"] = ob[p]

    in_maps = []
    for i in range(NCORES):
        shard = obs[i * BLOC:(i + 1) * BLOC, :D]
        xTa = np.concatenate(
            [np.ascontiguousarray(shard.T), np.ones((1, BLOC), np.float32)], axis=0
        )
        m = dict(consts)
        m["xTa"] = np.ascontiguousarray(xTa)
        in_maps.append(m)

    br = run_bass_kernel_spmd(nc, in_maps, list(range(NCORES)), trace=trace)
    LAST_RESULTS = br
    out = np.concatenate([br.results[i]["out"] for i in range(NCORES)], axis=0)
    return out.reshape(BATCH, NZ, NZ)



# revision 4
# speedup vs baseline: 2.3864x; 2.3864x over previous
"""Trainium2 Bass kernel for nn_CollectiveDecActorTaxi0Obs (gnn_message_passing).

Computes, for obs [32768, 48], per-zone dense heads W [81, 48, 5] (+bias b,
adjacency idx/mask [81, 5]):
    logits = einsum('bd,ndk->bnk', obs, W) + b ; masked softmax over k
    out[b, n, idx[n, k]] += probs[b, n, k]              -> [32768, 81, 81] f32

Strategy (pure data parallelism, 8 cores, batch-sharded 4096 rows each):
  The kernel is HBM-write-bound: the output is 860 MB dense but within the
  2e-2 tolerance, so the device writes it as bf16 (430 MB total, ~54 MB/core,
  ~150 us at the ~360 GB/s per-core DMA limit) and the host casts to f32.

  Everything runs with batch on the PARTITION dim in 32 sub-blocks of 128
  rows per core:
    - logits: one [49,128]^T @ [49,405] f32 matmul per sub-block (weights
      Wa pack all 81 zones' 5 slot columns + a bias row; masked slots get
      bias -1e9 so exp underflows to exactly 0).
    - exp on the scalar engine (PSUM -> SBUF), per-zone denominator via a
      window-5 avg-pool on DVE, then reciprocal (rc = 5/den; the extra 5
      folds into a scalar_tensor_tensor (e*0.2)*rc).
    - The scatter out[b, n, idx[n,k]] is batch-invariant: only ~405 of the
      6561 output columns are ever nonzero. Output tiles [128, 6561] bf16
      live persistently in SBUF, memset to zero ONCE; each sub-block just
      rewrites the hot columns with strided (e*0.2)*rc ops (dst stride 82
      on the zone-diagonal) split across DVE and GPSIMD, then DMAs the
      dense tile. Slot columns are class-assigned (self/left/right/up/down
      share a slot index across zones) so the whole scatter is 5 strided
      ops per sub-block (3D access patterns merge the per-grid-row runs).

  The host plans slot classes generically from idx/mask; if a zone has
  duplicate destinations (scatter-add semantics), it falls back to a dense
  scatter-matmul path (probs @ 0/1 S matrix, f32 output).
"""

import os
import sys

sys.path.insert(0, "/opt/trn_rl_repo")

import numpy as np

NZ = 81          # zones
D = 48           # obs dim used
DA = D + 1       # + bias row
KADJ = 5         # adjacency slots per zone
NCORES = 8
BATCH = 32768
BLOC = BATCH // NCORES   # 4096 rows per core
P = 128
NSUB = BLOC // P         # 32 sub-blocks of 128 batch rows
SLOTS = NZ * KADJ        # 405 packed slot columns
OUTW = NZ * NZ           # 6561 output columns
NOSB = 6                 # persistent output staging buffers
NEG = np.float32(-1e9)

LAST_RESULTS = None


# --------------------------------------------------------------------------
# Fast path: class-slot planning + strided-scatter program
# --------------------------------------------------------------------------

def _plan_scatter(idx, mask):
    """Assign each valid (zone, k) a slot class c so that zones sharing a
    destination offset o = idx-n share c, then group (o, c) classes into
    strided ops. Returns (assign, ops) or None if any zone has duplicate
    destinations (needs scatter-ADD, handled by the fallback path).

    assign: {n: {c: k}}   ops: [{o, c, z0, L, R, s}] meaning zones
    z0 + i*s + j for i<R, j<L write probs[:, 5*(z)+c] to out col 82*z + o.
    """
    from collections import Counter

    byzone = {}
    for n in range(NZ):
        dests = set()
        for k in range(KADJ):
            if mask[n, k] > 0:
                d = int(idx[n, k])
                if d in dests:
                    return None
                dests.add(d)
                byzone.setdefault(n, []).append((k, d - n))

    cnt = Counter(o for lst in byzone.values() for (_, o) in lst)
    pref = {o: r for r, (o, _) in enumerate(cnt.most_common())}

    assign = {n: {} for n in range(NZ)}
    classes = {}
    for n in range(NZ):
        used, rest = set(), []
        for k, o in sorted(byzone.get(n, []), key=lambda t: pref[t[1]]):
            c = pref[o]
            if c < KADJ and c not in used:
                used.add(c)
                assign[n][c] = k
                classes.setdefault((o, c), []).append(n)
            else:
                rest.append((k, o))
        free = [c for c in range(KADJ) if c not in used]
        for (k, o), c in zip(rest, free):
            assign[n][c] = k
            classes.setdefault((o, c), []).append(n)

    ops = []
    for (o, c), zones in sorted(classes.items()):
        zones.sort()
        runs, z0, prev = [], zones[0], zones[0]
        for z in zones[1:]:
            if z == prev + 1:
                prev = z
                continue
            runs.append((z0, prev - z0 + 1))
            z0 = prev = z
        runs.append((z0, prev - z0 + 1))
        if len(runs) >= 2:
            L = runs[0][1]
            s = runs[1][0] - runs[0][0]
            if (
                s > 0
                and all(r[1] == L for r in runs)
                and all(runs[i + 1][0] - runs[i][0] == s for i in range(len(runs) - 1))
            ):
                ops.append(dict(o=o, c=c, z0=runs[0][0], L=L, R=len(runs), s=s))
                continue
        for z0, L in runs:
            ops.append(dict(o=o, c=c, z0=z0, L=L, R=1, s=1))
    return assign, ops


def _build_wa(W, b, assign):
    W = np.asarray(W, np.float32)
    b = np.asarray(b, np.float32)
    Wa = np.zeros((DA, SLOTS), np.float32)
    Wa[D, :] = NEG                     # unassigned slots: exp -> exactly 0
    for n in range(NZ):
        for c, k in assign[n].items():
            col = KADJ * n + c
            Wa[:D, col] = W[n, :, k]
            Wa[D, col] = b[n, k]
    return Wa


def _build_program_fast(ops):
    from concourse import bacc, mybir
    from concourse.ap import AP
    import concourse.tile as tile

    f32 = mybir.dt.float32
    bf16 = mybir.dt.bfloat16
    AF = mybir.ActivationFunctionType
    OP = mybir.AluOpType
    nc = bacc.Bacc("TRN2", target_bir_lowering=False, debug=False)

    xTa_d = nc.declare_dram_parameter("xTa", [DA, BLOC], f32, isOutput=False)
    Wa_d = nc.declare_dram_parameter("Wa", [DA, SLOTS], f32, isOutput=False)
    out_d = nc.declare_dram_parameter("out", [BLOC, OUTW], bf16, isOutput=True)

    # Static DVE/GPSIMD split: greedily balance estimated per-sub-block time.
    # DVE starts pre-loaded with pool+reciprocal (~0.8 us equivalent).
    dve_t, gps_t = 805.0, 0.0
    eng_of = []
    for op in ops:
        elems = op["L"] * op["R"]
        dc = 130.0 + 1.05 * elems
        gc = 160.0 + 2.0 * elems
        if dve_t + dc <= gps_t + gc:
            eng_of.append(0)
            dve_t += dc
        else:
            eng_of.append(1)
            gps_t += gc

    with tile.TileContext(nc) as tc:
        with (
            tc.tile_pool(name="const", bufs=1) as cpool,
            tc.tile_pool(name="ework", bufs=4) as epool,
            tc.tile_pool(name="dwork", bufs=4) as dpool,
            tc.tile_pool(name="ps_lg", bufs=4, space="PSUM") as ps_lg,
        ):
            Wa_sb = cpool.tile([DA, SLOTS], f32, tag="Wa")
            nc.sync.dma_start(out=Wa_sb[:], in_=Wa_d[:])
            xTa_sb = cpool.tile([DA, BLOC], f32, tag="xTa")
            nc.sync.dma_start(out=xTa_sb[:], in_=xTa_d[:])

            osb = []
            for j in range(NOSB):
                t = cpool.tile([P, OUTW], bf16, tag=f"osb{j}")
                eng = nc.vector if j % 2 == 0 else nc.gpsimd
                eng.memset(t[:, :], 0.0)
                osb.append(t)

            for i in range(NSUB):
                lg = ps_lg.tile([P, SLOTS], f32, tag="lg")
                nc.tensor.matmul(
                    lg[:, :],
                    xTa_sb[:, i * P:(i + 1) * P],
                    Wa_sb[:, :],
                    start=True,
                    stop=True,
                )
                e = epool.tile([P, SLOTS], f32, tag="e")
                nc.scalar.activation(e[:, :], lg[:, :], AF.Exp)
                den = dpool.tile([P, NZ], f32, tag="den")
                nc.vector.tensor_reduce(
                    den[:, :],
                    e[:, :].rearrange("p (n k) -> p n k", k=KADJ),
                    mybir.AxisListType.X,
                    OP.add,
                )
                rc = dpool.tile([P, NZ], f32, tag="rc")
                nc.vector.reciprocal(rc[:, :], den[:, :])

                ot = osb[i % NOSB]
                et, rt, ott = e[:, :], rc[:, :], ot[:, :]
                for op, which in zip(ops, eng_of):
                    o, c, z0, L, R, s = (
                        op["o"], op["c"], op["z0"], op["L"], op["R"], op["s"],
                    )
                    src = AP(
                        tensor=et.tensor,
                        offset=KADJ * z0 + c,
                        ap=[[SLOTS, P], [KADJ * s, R], [KADJ, L]],
                    )
                    dst = AP(
                        tensor=ott.tensor,
                        offset=(NZ + 1) * z0 + o,
                        ap=[[OUTW, P], [(NZ + 1) * s, R], [NZ + 1, L]],
                    )
                    rca = AP(
                        tensor=rt.tensor,
                        offset=z0,
                        ap=[[NZ, P], [s, R], [1, L]],
                    )
                    eng = nc.vector if which == 0 else nc.gpsimd
                    eng.tensor_tensor(out=dst, in0=src, in1=rca, op=OP.mult)
                nc.sync.dma_start(
                    out=out_d[i * P:(i + 1) * P, :], in_=ot[:, :]
                )
    nc.compile()
    return nc


# --------------------------------------------------------------------------
# Fallback path (general scatter-add): dense scatter-matmul, f32 output
# --------------------------------------------------------------------------

BF = 512
ZPG = 6
NGRP = 14
GRP_NZ = [6] * 13 + [3]
GRP_COL = [486 * g for g in range(14)]
PW_PAIR = [128, 128, 128, 64]
PADW = 448


def _slot_mm(n, k):
    g = n // ZPG
    zz = n % ZPG
    hc = g // 2
    p = hc // 2
    row_hi = 32 * (g % 2) + KADJ * zz + k
    row_pair = 64 * (hc % 2) + row_hi
    return p, row_pair, hc, row_hi


def _build_consts_mm(W, b, idx, mask):
    import ml_dtypes

    bf = ml_dtypes.bfloat16
    W = np.asarray(W, np.float32)
    b = np.asarray(b, np.float32)
    idx = np.asarray(idx)
    mask = np.asarray(mask, np.float32)

    Wa = np.zeros((DA, PADW), np.float32)
    E = np.zeros((NZ, PADW), bf)
    ob = [np.zeros((PW_PAIR[p], NZ), np.float32) for p in range(4)]
    S = np.zeros((P, NZ * NZ), bf)

    for n in range(NZ):
        for k in range(KADJ):
            p, rp, hc, rh = _slot_mm(n, k)
            col = 128 * p + rp
            if mask[n, k] > 0:
                Wa[:D, col] = W[n, :, k]
                Wa[D, col] = b[n, k]
            else:
                Wa[D, col] = NEG
            E[n, col] = 1.0
            ob[p][rp, n] = 1.0
            ocol = n * NZ + int(idx[n, k])
            S[rh, ocol] = 1.0
            S[64 + rh, ocol] = 1.0
    return Wa, E, ob, S


def _build_program_mm(bloc):
    from concourse import bacc, mybir
    import concourse.tile as tile

    f32 = mybir.dt.float32
    bf16 = mybir.dt.bfloat16
    AF = mybir.ActivationFunctionType
    OP = mybir.AluOpType
    nc = bacc.Bacc("TRN2", target_bir_lowering=False, debug=False)

    xTa_d = nc.declare_dram_parameter("xTa", [DA, bloc], f32, isOutput=False)
    Wa_d = nc.declare_dram_parameter("Wa", [DA, PADW], f32, isOutput=False)
    E_d = nc.declare_dram_parameter("E", [NZ, PADW], bf16, isOutput=False)
    ob_d = [
        nc.declare_dram_parameter(f"ob{p}", [PW_PAIR[p], NZ], f32, isOutput=False)
        for p in range(4)
    ]
    S_d = nc.declare_dram_parameter("S", [P, NZ * NZ], bf16, isOutput=False)
    out_d = nc.declare_dram_parameter("out", [bloc, NZ * NZ], f32, isOutput=True)

    n_blk = bloc // BF
    n_sub = BF // P

    with tile.TileContext(nc) as tc:
        with (
            tc.tile_pool(name="const", bufs=1) as cpool,
            tc.tile_pool(name="work", bufs=2) as wpool,
            tc.tile_pool(name="outp", bufs=4) as opool,
            tc.tile_pool(name="ps_log", bufs=2, space="PSUM") as ps_log,
            tc.tile_pool(name="ps_den", bufs=1, space="PSUM") as ps_den,
            tc.tile_pool(name="ps_rf", bufs=2, space="PSUM") as ps_rf,
            tc.tile_pool(name="ps_sc", bufs=3, space="PSUM") as ps_sc,
        ):
            Wa_sb = cpool.tile([DA, PADW], f32, tag="Wa")
            nc.sync.dma_start(out=Wa_sb[:], in_=Wa_d[:])
            E_sb = cpool.tile([NZ, PADW], bf16, tag="E")
            nc.sync.dma_start(out=E_sb[:], in_=E_d[:])
            S_sb = cpool.tile([P, NZ * NZ], bf16, tag="S")
            nc.sync.dma_start(out=S_sb[:], in_=S_d[:])
            ob_sb = []
            for p in range(4):
                t = cpool.tile([PW_PAIR[p], NZ], f32, tag=f"ob{p}")
                nc.sync.dma_start(out=t[:], in_=ob_d[p][:])
                ob_sb.append(t)
            xTa_sb = cpool.tile([DA, bloc], f32, tag="xTa")
            nc.sync.dma_start(out=xTa_sb[:], in_=xTa_d[:])

            def emit_scatter(bs, pcat):
                for i in range(n_sub):
                    osb = opool.tile([P, NZ * NZ], f32, tag="osb")
                    for g in range(NGRP):
                        ncols = GRP_NZ[g] * NZ
                        colg = GRP_COL[g]
                        sc = ps_sc.tile([P, BF], f32, tag="scps")
                        nc.tensor.matmul(
                            sc[:, :ncols],
                            pcat[g // 2][:, i * P:(i + 1) * P],
                            S_sb[:, colg:colg + ncols],
                            start=True,
                            stop=True,
                        )
                        dst = osb[:, colg:colg + ncols]
                        if g % 5 < 3:
                            nc.scalar.copy(dst, sc[:, :ncols])
                        else:
                            nc.vector.tensor_copy(dst, sc[:, :ncols])
                    nc.sync.dma_start(
                        out=out_d[bs + i * P: bs + (i + 1) * P, :], in_=osb[:]
                    )

            prev = None
            for blk in range(n_blk):
                bs = blk * BF
                exT = []
                for p in range(4):
                    pw = PW_PAIR[p]
                    lg = ps_log.tile([P, BF], f32, tag="lg")
                    nc.tensor.matmul(
                        lg[:pw, :],
                        Wa_sb[:, 128 * p:128 * p + pw],
                        xTa_sb[:, bs:bs + BF],
                        start=True,
                        stop=True,
                    )
                    ex = wpool.tile([P, BF], f32, tag=f"exp{p}")
                    nc.scalar.activation(ex[:pw, :], lg[:pw, :], AF.Exp)
                    exT.append(ex)
                den_ps = ps_den.tile([NZ, BF], f32, tag="den")
                for p in range(4):
                    nc.tensor.matmul(
                        den_ps[:, :], ob_sb[p][:], exT[p][:PW_PAIR[p], :],
                        start=(p == 0), stop=(p == 3),
                    )
                rc = wpool.tile([NZ, BF], f32, tag="recipC")
                nc.vector.reciprocal(rc[:], den_ps[:])
                rhi = wpool.tile([NZ, BF], bf16, tag="rhi")
                nc.scalar.copy(rhi[:], rc[:])
                rlo = wpool.tile([NZ, BF], bf16, tag="rlo")
                nc.vector.tensor_tensor(out=rlo[:], in0=rc[:], in1=rhi[:], op=OP.subtract)
                pcat = []
                for p in range(4):
                    pw = PW_PAIR[p]
                    rf = ps_rf.tile([P, BF], f32, tag="rf")
                    nc.tensor.matmul(
                        rf[:pw, :], E_sb[:, 128 * p:128 * p + pw], rhi[:],
                        start=True, stop=False,
                    )
                    nc.tensor.matmul(
                        rf[:pw, :], E_sb[:, 128 * p:128 * p + pw], rlo[:],
                        start=False, stop=True,
                    )
                    for h in range(2 if pw == 128 else 1):
                        sl = slice(64 * h, 64 * h + 64)
                        pt = wpool.tile([64, BF], f32, tag=f"pt{2 * p + h}")
                        nc.vector.tensor_tensor(
                            out=pt[:, :], in0=exT[p][sl, :], in1=rf[sl, :], op=OP.mult
                        )
                        pc = wpool.tile([P, BF], bf16, tag=f"pcat{2 * p + h}")
                        nc.scalar.copy(pc[:64, :], pt[:, :])
                        nc.vector.tensor_tensor(
                            out=pc[64:, :],
                            in0=pt[:, :],
                            in1=pc[:64, :],
                            op=OP.subtract,
                        )
                        pcat.append(pc)
                if prev is not None:
                    emit_scatter(*prev)
                prev = (bs, pcat)
            emit_scatter(*prev)
    nc.compile()
    return nc


# --------------------------------------------------------------------------
# Entry
# --------------------------------------------------------------------------

def _install_ntff_hook():
    """Shim antenv.axon_hooks (absent in this image) so trace=True can drive
    NRT profiling through libaxon_pjrt.so. Only used for self-profiling."""
    import types

    try:
        import antenv

        try:
            from antenv.axon_hooks import get_axon_ntff_profile_hook  # noqa: F401

            return True
        except ImportError:
            pass
        if "/root/.axon_site" not in sys.path:
            sys.path.insert(0, "/root/.axon_site")
        from trn_agent_boot.trn_boot import _ntff_profile_via_ctypes

        hook = _ntff_profile_via_ctypes("/opt/axon/libaxon_pjrt.so")
        mod = types.ModuleType("antenv.axon_hooks")
        state = {"hook": hook}
        mod.get_axon_ntff_profile_hook = lambda: state["hook"]
        mod.set_axon_ntff_profile_hook = lambda h: state.update(hook=h)
        sys.modules["antenv.axon_hooks"] = mod
        antenv.axon_hooks = mod
        return hook is not None
    except Exception as e:  # profiling is best-effort; never break the run
        print("ntff hook install failed:", e)
        return False


def _make_xta_maps(obs, consts):
    in_maps = []
    for i in range(NCORES):
        shard = obs[i * BLOC:(i + 1) * BLOC, :D]
        xTa = np.concatenate(
            [np.ascontiguousarray(shard.T), np.ones((1, BLOC), np.float32)], axis=0
        )
        m = dict(consts)
        m["xTa"] = np.ascontiguousarray(xTa)
        in_maps.append(m)
    return in_maps


def kernel(obs, W, b, idx, mask):
    from concourse.bass_utils import run_bass_kernel_spmd

    global LAST_RESULTS
    trace = bool(int(os.environ.get("KBT_TRACE", "0")))
    if trace:
        trace = _install_ntff_hook()
    obs = np.asarray(obs, np.float32)
    idx = np.asarray(idx)
    mask = np.asarray(mask, np.float32)

    plan = _plan_scatter(idx, mask)
    if plan is not None:
        assign, ops = plan
        Wa = _build_wa(W, b, assign)
        nc = _build_program_fast(ops)
        in_maps = _make_xta_maps(obs, {"Wa": Wa})
        br = run_bass_kernel_spmd(nc, in_maps, list(range(NCORES)), trace=trace)
        LAST_RESULTS = br
        out = np.concatenate(
            [
                np.asarray(br.results[i]["out"]).astype(np.float32)
                for i in range(NCORES)
            ],
            axis=0,
        )
        return out.reshape(BATCH, NZ, NZ)

    # general scatter-add fallback
    Wa, E, ob, S = _build_consts_mm(W, b, idx, mask)
    nc = _build_program_mm(BLOC)
    consts = {"Wa": Wa, "E": E, "S": S}
    for p in range(4):
        consts[f"ob{p}"] = ob[p]
    in_maps = _make_xta_maps(obs, consts)
    br = run_bass_kernel_spmd(nc, in_maps, list(range(NCORES)), trace=trace)
    LAST_RESULTS = br
    out = np.concatenate([br.results[i]["out"] for i in range(NCORES)], axis=0)
    return out.reshape(BATCH, NZ, NZ)


# revision 12
# speedup vs baseline: 2.6165x; 1.0964x over previous
"""Trainium2 Bass kernel for nn_CollectiveDecActorTaxi0Obs (gnn_message_passing).

Computes, for obs [32768, 48], per-zone dense heads W [81, 48, 5] (+bias b,
adjacency idx/mask [81, 5]):
    logits = einsum('bd,ndk->bnk', obs, W) + b ; masked softmax over k
    out[b, n, idx[n, k]] += probs[b, n, k]              -> [32768, 81, 81] f32

Strategy (pure data parallelism, 8 cores, batch-sharded 4096 rows each):
  The kernel is HBM-write-bound: the output is 860 MB dense but within the
  2e-2 tolerance, so the device writes it as bf16 (430 MB total, ~54 MB/core,
  ~150 us at the ~360 GB/s per-core DMA limit) and the host casts to f32.

  Everything runs with batch on the PARTITION dim in 32 sub-blocks of 128
  rows per core:
    - logits: one [49,128]^T @ [49,405] f32 matmul per sub-block (weights
      Wa pack all 81 zones' 5 slot columns + a bias row; masked slots get
      bias -1e9 so exp underflows to exactly 0).
    - exp on the scalar engine (PSUM -> SBUF), per-zone denominator via a
      window-5 avg-pool on DVE, then reciprocal (rc = 5/den; the extra 5
      folds into a scalar_tensor_tensor (e*0.2)*rc).
    - The scatter out[b, n, idx[n,k]] is batch-invariant: only ~405 of the
      6561 output columns are ever nonzero. Output tiles [128, 6561] bf16
      live persistently in SBUF, memset to zero ONCE; each sub-block just
      rewrites the hot columns with strided (e*0.2)*rc ops (dst stride 82
      on the zone-diagonal) split across DVE and GPSIMD, then DMAs the
      dense tile. Slot columns are class-assigned (self/left/right/up/down
      share a slot index across zones) so the whole scatter is 5 strided
      ops per sub-block (3D access patterns merge the per-grid-row runs).

  The host plans slot classes generically from idx/mask; if a zone has
  duplicate destinations (scatter-add semantics), it falls back to a dense
  scatter-matmul path (probs @ 0/1 S matrix, f32 output).
"""

import os
import sys

sys.path.insert(0, "/opt/trn_rl_repo")

import numpy as np

NZ = 81          # zones
D = 48           # obs dim used
DA = D + 1       # + bias row
KADJ = 5         # adjacency slots per zone
NCORES = 8
BATCH = 32768
BLOC = BATCH // NCORES   # 4096 rows per core
P = 128
NSUB = BLOC // P         # 32 sub-blocks of 128 batch rows
SLOTS = NZ * KADJ        # 405 packed slot columns
OUTW = NZ * NZ           # 6561 output columns
NOSB = 6                 # persistent output staging buffers
NEG = np.float32(-1e9)

LAST_RESULTS = None


# --------------------------------------------------------------------------
# Fast path: class-slot planning + strided-scatter program
# --------------------------------------------------------------------------

def _plan_scatter(idx, mask):
    """Assign each valid (zone, k) a slot class c so that zones sharing a
    destination offset o = idx-n share c, then group (o, c) classes into
    strided ops. Returns (assign, ops) or None if any zone has duplicate
    destinations (needs scatter-ADD, handled by the fallback path).

    assign: {n: {c: k}}   ops: [{o, c, z0, L, R, s}] meaning zones
    z0 + i*s + j for i<R, j<L write probs[:, 5*(z)+c] to out col 82*z + o.
    """
    from collections import Counter

    byzone = {}
    for n in range(NZ):
        dests = set()
        for k in range(KADJ):
            if mask[n, k] > 0:
                d = int(idx[n, k])
                if d in dests:
                    return None
                dests.add(d)
                byzone.setdefault(n, []).append((k, d - n))

    cnt = Counter(o for lst in byzone.values() for (_, o) in lst)
    pref = {o: r for r, (o, _) in enumerate(cnt.most_common())}

    assign = {n: {} for n in range(NZ)}
    offs = {n: set(o for (_, o) in byzone.get(n, [])) for n in range(NZ)}
    classes = {}
    for n in range(NZ):
        used, rest = set(), []
        for k, o in sorted(byzone.get(n, []), key=lambda t: pref[t[1]]):
            c = pref[o]
            if c < KADJ and c not in used:
                used.add(c)
                assign[n][c] = k
                classes.setdefault((o, c), []).append(n)
            else:
                rest.append((k, o))
        free = [c for c in range(KADJ) if c not in used]
        for (k, o), c in zip(rest, free):
            assign[n][c] = k
            classes.setdefault((o, c), []).append(n)

    ops = []
    for (o, c), zones in sorted(classes.items()):
        zones.sort()
        zone_set = set(zones)
        # Try a single contiguous run over the class hull: gap zones are
        # safe to include when slot c is unassigned there (the Wa column
        # gets bias -1e9, exp -> 0, so we write a computed 0 to a column
        # that is permanently zero) — requires the gap zone to have no
        # OTHER slot with this same offset (would race with its writer),
        # at least one valid slot (else rc is inf -> 0*inf = NaN), and
        # the destination column in bounds.
        a, b = zones[0], zones[-1]
        n_min = max(0, (-o + NZ) // (NZ + 1))
        n_max = min(NZ - 1, (OUTW - 1 - o) // (NZ + 1))
        hull_ok = a >= n_min and b <= n_max and all(
            z in zone_set
            or (c not in assign[z] and o not in offs[z] and byzone.get(z))
            for z in range(a, b + 1)
        )
        if hull_ok:
            ops.append(dict(o=o, c=c, z0=a, L=b - a + 1, R=1, s=1))
            continue
        runs, z0, prev = [], zones[0], zones[0]
        for z in zones[1:]:
            if z == prev + 1:
                prev = z
                continue
            runs.append((z0, prev - z0 + 1))
            z0 = prev = z
        runs.append((z0, prev - z0 + 1))
        if len(runs) >= 2:
            L = runs[0][1]
            s = runs[1][0] - runs[0][0]
            if (
                s > 0
                and all(r[1] == L for r in runs)
                and all(runs[i + 1][0] - runs[i][0] == s for i in range(len(runs) - 1))
            ):
                ops.append(dict(o=o, c=c, z0=runs[0][0], L=L, R=len(runs), s=s))
                continue
        for z0, L in runs:
            ops.append(dict(o=o, c=c, z0=z0, L=L, R=1, s=1))
    return assign, ops


def _build_wa(W, b, assign):
    W = np.asarray(W, np.float32)
    b = np.asarray(b, np.float32)
    Wa = np.zeros((DA, SLOTS), np.float32)
    Wa[D, :] = NEG                     # unassigned slots: exp -> exactly 0
    for n in range(NZ):
        for c, k in assign[n].items():
            col = KADJ * n + c
            Wa[:D, col] = W[n, :, k]
            Wa[D, col] = b[n, k]
    return Wa


def _build_program_fast(ops):
    from concourse import bacc, mybir
    from concourse.ap import AP
    import concourse.tile as tile

    f32 = mybir.dt.float32
    f32r = mybir.dt.float32r
    bf16 = mybir.dt.bfloat16
    AF = mybir.ActivationFunctionType
    OP = mybir.AluOpType
    nc = bacc.Bacc("TRN2", target_bir_lowering=False, debug=False)

    xTa_d = nc.declare_dram_parameter("xTa", [DA, BLOC], f32, isOutput=False)
    Wa_d = nc.declare_dram_parameter("Wa", [DA, SLOTS], f32, isOutput=False)
    out_d = nc.declare_dram_parameter("out", [BLOC, OUTW], bf16, isOutput=True)

    # Static DVE/GPSIMD split: greedily balance estimated per-sub-block time.
    # DVE starts pre-loaded with pool+reciprocal (~0.8 us equivalent).
    dve_t, gps_t = 805.0, 0.0
    eng_of = []
    for op in ops:
        elems = op["L"] * op["R"]
        dc = 130.0 + 1.05 * elems
        gc = 160.0 + 2.0 * elems
        if dve_t + dc <= gps_t + gc:
            eng_of.append(0)
            dve_t += dc
        else:
            eng_of.append(1)
            gps_t += gc

    with tile.TileContext(nc) as tc:
        with (
            tc.tile_pool(name="const", bufs=1) as cpool,
            tc.tile_pool(name="ework", bufs=4) as epool,
            tc.tile_pool(name="dwork", bufs=4) as dpool,
            tc.tile_pool(name="ps_lg", bufs=4, space="PSUM") as ps_lg,
        ):
            Wa_sb = cpool.tile([DA, SLOTS], f32, tag="Wa")
            nc.sync.dma_start(out=Wa_sb[:], in_=Wa_d[:])
            xTa_sb = cpool.tile([DA, BLOC], f32, tag="xTa")
            # chunked load so sub-block 0's matmul starts early
            XCH = 512
            for j in range(BLOC // XCH):
                nc.sync.dma_start(
                    out=xTa_sb[:, j * XCH:(j + 1) * XCH],
                    in_=xTa_d[:, j * XCH:(j + 1) * XCH],
                )

            # persistent output tiles; memsets interleaved with the first
            # sub-blocks on gpsimd (DVE memsets are 2x slower and would
            # stall the first scatters behind them)
            osb = []
            for j in range(NOSB):
                ot = cpool.tile([P, OUTW], bf16, tag=f"osb{j}")
                osb.append(ot)

            for i in range(NSUB):
                if i < NOSB:
                    nc.gpsimd.memset(osb[i][:, :], 0.0)
                lg = ps_lg.tile([P, SLOTS], f32, tag="lg")
                nc.tensor.matmul(
                    lg[:, :],
                    xTa_sb[:, i * P:(i + 1) * P],
                    Wa_sb[:, :],
                    start=True,
                    stop=True,
                )
                e = epool.tile([P, SLOTS], f32, tag="e")
                nc.scalar.activation(e[:, :], lg[:, :], AF.Exp)
                den = dpool.tile([P, NZ], f32, tag="den")
                nc.vector.tensor_reduce(
                    den[:, :],
                    e[:, :].rearrange("p (n k) -> p n k", k=KADJ),
                    mybir.AxisListType.X,
                    OP.add,
                )
                rc = dpool.tile([P, NZ], f32, tag="rc")
                nc.vector.reciprocal(rc[:, :], den[:, :])

                ot = osb[i % NOSB]
                et, rt, ott = e[:, :], rc[:, :], ot[:, :]
                for op, which in zip(ops, eng_of):
                    o, c, z0, L, R, s = (
                        op["o"], op["c"], op["z0"], op["L"], op["R"], op["s"],
                    )
                    src = AP(
                        tensor=et.tensor,
                        offset=KADJ * z0 + c,
                        ap=[[SLOTS, P], [KADJ * s, R], [KADJ, L]],
                    )
                    dst = AP(
                        tensor=ott.tensor,
                        offset=(NZ + 1) * z0 + o,
                        ap=[[OUTW, P], [(NZ + 1) * s, R], [NZ + 1, L]],
                    )
                    rca = AP(
                        tensor=rt.tensor,
                        offset=z0,
                        ap=[[NZ, P], [s, R], [1, L]],
                    )
                    eng = nc.vector if which == 0 else nc.gpsimd
                    eng.tensor_tensor(out=dst, in0=src, in1=rca, op=OP.mult)
                nc.sync.dma_start(
                    out=out_d[i * P:(i + 1) * P, :], in_=ot[:, :]
                )
    nc.compile()
    return nc


# --------------------------------------------------------------------------
# Fallback path (general scatter-add): dense scatter-matmul, f32 output
# --------------------------------------------------------------------------

BF = 512
ZPG = 6
NGRP = 14
GRP_NZ = [6] * 13 + [3]
GRP_COL = [486 * g for g in range(14)]
PW_PAIR = [128, 128, 128, 64]
PADW = 448


def _slot_mm(n, k):
    g = n // ZPG
    zz = n % ZPG
    hc = g // 2
    p = hc // 2
    row_hi = 32 * (g % 2) + KADJ * zz + k
    row_pair = 64 * (hc % 2) + row_hi
    return p, row_pair, hc, row_hi


def _build_consts_mm(W, b, idx, mask):
    import ml_dtypes

    bf = ml_dtypes.bfloat16
    W = np.asarray(W, np.float32)
    b = np.asarray(b, np.float32)
    idx = np.asarray(idx)
    mask = np.asarray(mask, np.float32)

    Wa = np.zeros((DA, PADW), np.float32)
    E = np.zeros((NZ, PADW), bf)
    ob = [np.zeros((PW_PAIR[p], NZ), np.float32) for p in range(4)]
    S = np.zeros((P, NZ * NZ), bf)

    for n in range(NZ):
        for k in range(KADJ):
            p, rp, hc, rh = _slot_mm(n, k)
            col = 128 * p + rp
            if mask[n, k] > 0:
                Wa[:D, col] = W[n, :, k]
                Wa[D, col] = b[n, k]
            else:
                Wa[D, col] = NEG
            E[n, col] = 1.0
            ob[p][rp, n] = 1.0
            ocol = n * NZ + int(idx[n, k])
            S[rh, ocol] = 1.0
            S[64 + rh, ocol] = 1.0
    return Wa, E, ob, S


def _build_program_mm(bloc):
    from concourse import bacc, mybir
    import concourse.tile as tile

    f32 = mybir.dt.float32
    bf16 = mybir.dt.bfloat16
    AF = mybir.ActivationFunctionType
    OP = mybir.AluOpType
    nc = bacc.Bacc("TRN2", target_bir_lowering=False, debug=False)

    xTa_d = nc.declare_dram_parameter("xTa", [DA, bloc], f32, isOutput=False)
    Wa_d = nc.declare_dram_parameter("Wa", [DA, PADW], f32, isOutput=False)
    E_d = nc.declare_dram_parameter("E", [NZ, PADW], bf16, isOutput=False)
    ob_d = [
        nc.declare_dram_parameter(f"ob{p}", [PW_PAIR[p], NZ], f32, isOutput=False)
        for p in range(4)
    ]
    S_d = nc.declare_dram_parameter("S", [P, NZ * NZ], bf16, isOutput=False)
    out_d = nc.declare_dram_parameter("out", [bloc, NZ * NZ], f32, isOutput=True)

    n_blk = bloc // BF
    n_sub = BF // P

    with tile.TileContext(nc) as tc:
        with (
            tc.tile_pool(name="const", bufs=1) as cpool,
            tc.tile_pool(name="work", bufs=2) as wpool,
            tc.tile_pool(name="outp", bufs=4) as opool,
            tc.tile_pool(name="ps_log", bufs=2, space="PSUM") as ps_log,
            tc.tile_pool(name="ps_den", bufs=1, space="PSUM") as ps_den,
            tc.tile_pool(name="ps_rf", bufs=2, space="PSUM") as ps_rf,
            tc.tile_pool(name="ps_sc", bufs=3, space="PSUM") as ps_sc,
        ):
            Wa_sb = cpool.tile([DA, PADW], f32, tag="Wa")
            nc.sync.dma_start(out=Wa_sb[:], in_=Wa_d[:])
            E_sb = cpool.tile([NZ, PADW], bf16, tag="E")
            nc.sync.dma_start(out=E_sb[:], in_=E_d[:])
            S_sb = cpool.tile([P, NZ * NZ], bf16, tag="S")
            nc.sync.dma_start(out=S_sb[:], in_=S_d[:])
            ob_sb = []
            for p in range(4):
                t = cpool.tile([PW_PAIR[p], NZ], f32, tag=f"ob{p}")
                nc.sync.dma_start(out=t[:], in_=ob_d[p][:])
                ob_sb.append(t)
            xTa_sb = cpool.tile([DA, bloc], f32, tag="xTa")
            nc.sync.dma_start(out=xTa_sb[:], in_=xTa_d[:])

            def emit_scatter(bs, pcat):
                for i in range(n_sub):
                    osb = opool.tile([P, NZ * NZ], f32, tag="osb")
                    for g in range(NGRP):
                        ncols = GRP_NZ[g] * NZ
                        colg = GRP_COL[g]
                        sc = ps_sc.tile([P, BF], f32, tag="scps")
                        nc.tensor.matmul(
                            sc[:, :ncols],
                            pcat[g // 2][:, i * P:(i + 1) * P],
                            S_sb[:, colg:colg + ncols],
                            start=True,
                            stop=True,
                        )
                        dst = osb[:, colg:colg + ncols]
                        if g % 5 < 3:
                            nc.scalar.copy(dst, sc[:, :ncols])
                        else:
                            nc.vector.tensor_copy(dst, sc[:, :ncols])
                    nc.sync.dma_start(
                        out=out_d[bs + i * P: bs + (i + 1) * P, :], in_=osb[:]
                    )

            prev = None
            for blk in range(n_blk):
                bs = blk * BF
                exT = []
                for p in range(4):
                    pw = PW_PAIR[p]
                    lg = ps_log.tile([P, BF], f32, tag="lg")
                    nc.tensor.matmul(
                        lg[:pw, :],
                        Wa_sb[:, 128 * p:128 * p + pw],
                        xTa_sb[:, bs:bs + BF],
                        start=True,
                        stop=True,
                    )
                    ex = wpool.tile([P, BF], f32, tag=f"exp{p}")
                    nc.scalar.activation(ex[:pw, :], lg[:pw, :], AF.Exp)
                    exT.append(ex)
                den_ps = ps_den.tile([NZ, BF], f32, tag="den")
                for p in range(4):
                    nc.tensor.matmul(
                        den_ps[:, :], ob_sb[p][:], exT[p][:PW_PAIR[p], :],
                        start=(p == 0), stop=(p == 3),
                    )
                rc = wpool.tile([NZ, BF], f32, tag="recipC")
                nc.vector.reciprocal(rc[:], den_ps[:])
                rhi = wpool.tile([NZ, BF], bf16, tag="rhi")
                nc.scalar.copy(rhi[:], rc[:])
                rlo = wpool.tile([NZ, BF], bf16, tag="rlo")
                nc.vector.tensor_tensor(out=rlo[:], in0=rc[:], in1=rhi[:], op=OP.subtract)
                pcat = []
                for p in range(4):
                    pw = PW_PAIR[p]
                    rf = ps_rf.tile([P, BF], f32, tag="rf")
                    nc.tensor.matmul(
                        rf[:pw, :], E_sb[:, 128 * p:128 * p + pw], rhi[:],
                        start=True, stop=False,
                    )
                    nc.tensor.matmul(
                        rf[:pw, :], E_sb[:, 128 * p:128 * p + pw], rlo[:],
                        start=False, stop=True,
                    )
                    for h in range(2 if pw == 128 else 1):
                        sl = slice(64 * h, 64 * h + 64)
                        pt = wpool.tile([64, BF], f32, tag=f"pt{2 * p + h}")
                        nc.vector.tensor_tensor(
                            out=pt[:, :], in0=exT[p][sl, :], in1=rf[sl, :], op=OP.mult
                        )
                        pc = wpool.tile([P, BF], bf16, tag=f"pcat{2 * p + h}")
                        nc.scalar.copy(pc[:64, :], pt[:, :])
                        nc.vector.tensor_tensor(
                            out=pc[64:, :],
                            in0=pt[:, :],
                            in1=pc[:64, :],
                            op=OP.subtract,
                        )
                        pcat.append(pc)
                if prev is not None:
                    emit_scatter(*prev)
                prev = (bs, pcat)
            emit_scatter(*prev)
    nc.compile()
    return nc


# --------------------------------------------------------------------------
# Entry
# --------------------------------------------------------------------------

def _install_ntff_hook():
    """Shim antenv.axon_hooks (absent in this image) so trace=True can drive
    NRT profiling through libaxon_pjrt.so. Only used for self-profiling."""
    import types

    try:
        import antenv

        try:
            from antenv.axon_hooks import get_axon_ntff_profile_hook  # noqa: F401

            return True
        except ImportError:
            pass
        if "/root/.axon_site" not in sys.path:
            sys.path.insert(0, "/root/.axon_site")
        from trn_agent_boot.trn_boot import _ntff_profile_via_ctypes

        hook = _ntff_profile_via_ctypes("/opt/axon/libaxon_pjrt.so")
        mod = types.ModuleType("antenv.axon_hooks")
        state = {"hook": hook}
        mod.get_axon_ntff_profile_hook = lambda: state["hook"]
        mod.set_axon_ntff_profile_hook = lambda h: state.update(hook=h)
        sys.modules["antenv.axon_hooks"] = mod
        antenv.axon_hooks = mod
        return hook is not None
    except Exception as e:  # profiling is best-effort; never break the run
        print("ntff hook install failed:", e)
        return False


def _make_xta_maps(obs, consts):
    in_maps = []
    for i in range(NCORES):
        shard = obs[i * BLOC:(i + 1) * BLOC, :D]
        xTa = np.concatenate(
            [np.ascontiguousarray(shard.T), np.ones((1, BLOC), np.float32)], axis=0
        )
        m = dict(consts)
        m["xTa"] = np.ascontiguousarray(xTa)
        in_maps.append(m)
    return in_maps


def kernel(obs, W, b, idx, mask):
    from concourse.bass_utils import run_bass_kernel_spmd

    global LAST_RESULTS
    trace = bool(int(os.environ.get("KBT_TRACE", "0")))
    if trace:
        trace = _install_ntff_hook()
    obs = np.asarray(obs, np.float32)
    idx = np.asarray(idx)
    mask = np.asarray(mask, np.float32)

    plan = _plan_scatter(idx, mask)
    if plan is not None:
        assign, ops = plan
        Wa = _build_wa(W, b, assign)
        nc = _build_program_fast(ops)
        in_maps = _make_xta_maps(obs, {"Wa": Wa})
        br = run_bass_kernel_spmd(nc, in_maps, list(range(NCORES)), trace=trace)
        LAST_RESULTS = br
        out = np.concatenate(
            [
                np.asarray(br.results[i]["out"]).astype(np.float32)
                for i in range(NCORES)
            ],
            axis=0,
        )
        return out.reshape(BATCH, NZ, NZ)

    # general scatter-add fallback
    Wa, E, ob, S = _build_consts_mm(W, b, idx, mask)
    nc = _build_program_mm(BLOC)
    consts = {"Wa": Wa, "E": E, "S": S}
    for p in range(4):
        consts[f"ob{p}"] = ob[p]
    in_maps = _make_xta_maps(obs, consts)
    br = run_bass_kernel_spmd(nc, in_maps, list(range(NCORES)), trace=trace)
    LAST_RESULTS = br
    out = np.concatenate([br.results[i]["out"] for i in range(NCORES)], axis=0)
    return out.reshape(BATCH, NZ, NZ)


# revision 14
# speedup vs baseline: 2.7064x; 1.0343x over previous
"""Trainium2 Bass kernel for nn_CollectiveDecActorTaxi0Obs (gnn_message_passing).

Computes, for obs [32768, 48], per-zone dense heads W [81, 48, 5] (+bias b,
adjacency idx/mask [81, 5]):
    logits = einsum('bd,ndk->bnk', obs, W) + b ; masked softmax over k
    out[b, n, idx[n, k]] += probs[b, n, k]              -> [32768, 81, 81] f32

Strategy (pure data parallelism, 8 cores, batch-sharded 4096 rows each):
  The kernel is HBM-write-bound: the output is 860 MB dense but within the
  2e-2 tolerance, so the device writes it as bf16 (430 MB total, ~54 MB/core,
  ~150 us at the ~360 GB/s per-core DMA limit) and the host casts to f32.

  Everything runs with batch on the PARTITION dim in 32 sub-blocks of 128
  rows per core:
    - logits: one [49,128]^T @ [49,405] f32 matmul per sub-block (weights
      Wa pack all 81 zones' 5 slot columns + a bias row; masked slots get
      bias -1e9 so exp underflows to exactly 0).
    - exp on the scalar engine (PSUM -> SBUF), per-zone denominator via a
      window-5 avg-pool on DVE, then reciprocal (rc = 5/den; the extra 5
      folds into a scalar_tensor_tensor (e*0.2)*rc).
    - The scatter out[b, n, idx[n,k]] is batch-invariant: only ~405 of the
      6561 output columns are ever nonzero. Output tiles [128, 6561] bf16
      live persistently in SBUF, memset to zero ONCE; each sub-block just
      rewrites the hot columns with strided (e*0.2)*rc ops (dst stride 82
      on the zone-diagonal) split across DVE and GPSIMD, then DMAs the
      dense tile. Slot columns are class-assigned (self/left/right/up/down
      share a slot index across zones) so the whole scatter is 5 strided
      ops per sub-block (3D access patterns merge the per-grid-row runs).

  The host plans slot classes generically from idx/mask; if a zone has
  duplicate destinations (scatter-add semantics), it falls back to a dense
  scatter-matmul path (probs @ 0/1 S matrix, f32 output).
"""

import os
import sys

sys.path.insert(0, "/opt/trn_rl_repo")

import numpy as np

NZ = 81          # zones
D = 48           # obs dim used
DA = D + 1       # + bias row
KADJ = 5         # adjacency slots per zone
NCORES = 8
BATCH = 32768
BLOC = BATCH // NCORES   # 4096 rows per core
P = 128
NSUB = BLOC // P         # 32 sub-blocks of 128 batch rows
SLOTS = NZ * KADJ        # 405 packed slot columns
OUTW = NZ * NZ           # 6561 output columns
NOSB = 4                 # persistent output staging buffers
NEG = np.float32(-1e9)

LAST_RESULTS = None


# --------------------------------------------------------------------------
# Fast path: class-slot planning + strided-scatter program
# --------------------------------------------------------------------------

def _plan_scatter(idx, mask):
    """Assign each valid (zone, k) a slot class c so that zones sharing a
    destination offset o = idx-n share c, then group (o, c) classes into
    strided ops. Returns (assign, ops) or None if any zone has duplicate
    destinations (needs scatter-ADD, handled by the fallback path).

    assign: {n: {c: k}}   ops: [{o, c, z0, L, R, s}] meaning zones
    z0 + i*s + j for i<R, j<L write probs[:, 5*(z)+c] to out col 82*z + o.
    """
    from collections import Counter

    byzone = {}
    for n in range(NZ):
        dests = set()
        for k in range(KADJ):
            if mask[n, k] > 0:
                d = int(idx[n, k])
                if d in dests:
                    return None
                dests.add(d)
                byzone.setdefault(n, []).append((k, d - n))

    cnt = Counter(o for lst in byzone.values() for (_, o) in lst)
    pref = {o: r for r, (o, _) in enumerate(cnt.most_common())}

    assign = {n: {} for n in range(NZ)}
    offs = {n: set(o for (_, o) in byzone.get(n, [])) for n in range(NZ)}
    classes = {}
    for n in range(NZ):
        used, rest = set(), []
        for k, o in sorted(byzone.get(n, []), key=lambda t: pref[t[1]]):
            c = pref[o]
            if c < KADJ and c not in used:
                used.add(c)
                assign[n][c] = k
                classes.setdefault((o, c), []).append(n)
            else:
                rest.append((k, o))
        free = [c for c in range(KADJ) if c not in used]
        for (k, o), c in zip(rest, free):
            assign[n][c] = k
            classes.setdefault((o, c), []).append(n)

    ops = []
    for (o, c), zones in sorted(classes.items()):
        zones.sort()
        zone_set = set(zones)
        # Try a single contiguous run over the class hull: gap zones are
        # safe to include when slot c is unassigned there (the Wa column
        # gets bias -1e9, exp -> 0, so we write a computed 0 to a column
        # that is permanently zero) — requires the gap zone to have no
        # OTHER slot with this same offset (would race with its writer),
        # at least one valid slot (else rc is inf -> 0*inf = NaN), and
        # the destination column in bounds.
        a, b = zones[0], zones[-1]
        n_min = max(0, (-o + NZ) // (NZ + 1))
        n_max = min(NZ - 1, (OUTW - 1 - o) // (NZ + 1))
        hull_ok = a >= n_min and b <= n_max and all(
            z in zone_set
            or (c not in assign[z] and o not in offs[z] and byzone.get(z))
            for z in range(a, b + 1)
        )
        if hull_ok:
            ops.append(dict(o=o, c=c, z0=a, L=b - a + 1, R=1, s=1))
            continue
        runs, z0, prev = [], zones[0], zones[0]
        for z in zones[1:]:
            if z == prev + 1:
                prev = z
                continue
            runs.append((z0, prev - z0 + 1))
            z0 = prev = z
        runs.append((z0, prev - z0 + 1))
        if len(runs) >= 2:
            L = runs[0][1]
            s = runs[1][0] - runs[0][0]
            if (
                s > 0
                and all(r[1] == L for r in runs)
                and all(runs[i + 1][0] - runs[i][0] == s for i in range(len(runs) - 1))
            ):
                ops.append(dict(o=o, c=c, z0=runs[0][0], L=L, R=len(runs), s=s))
                continue
        for z0, L in runs:
            ops.append(dict(o=o, c=c, z0=z0, L=L, R=1, s=1))
    return assign, ops


def _build_wa(W, b, assign):
    W = np.asarray(W, np.float32)
    b = np.asarray(b, np.float32)
    Wa = np.zeros((DA, SLOTS), np.float32)
    Wa[D, :] = NEG                     # unassigned slots: exp -> exactly 0
    for n in range(NZ):
        for c, k in assign[n].items():
            col = KADJ * n + c
            Wa[:D, col] = W[n, :, k]
            Wa[D, col] = b[n, k]
    return Wa


def _build_program_fast(ops):
    from concourse import bacc, mybir
    from concourse.ap import AP
    import concourse.tile as tile

    f32 = mybir.dt.float32
    f32r = mybir.dt.float32r
    bf16 = mybir.dt.bfloat16
    AF = mybir.ActivationFunctionType
    OP = mybir.AluOpType
    nc = bacc.Bacc("TRN2", target_bir_lowering=False, debug=False)

    xTa_d = nc.declare_dram_parameter("xTa", [DA, BLOC], f32, isOutput=False)
    Wa_d = nc.declare_dram_parameter("Wa", [DA, SLOTS], f32, isOutput=False)
    out_d = nc.declare_dram_parameter("out", [BLOC, OUTW], bf16, isOutput=True)

    # Static DVE/GPSIMD split: greedily balance estimated per-sub-block time.
    # DVE starts pre-loaded with pool+reciprocal (~0.8 us equivalent).
    dve_t, gps_t = 805.0, 0.0
    eng_of = []
    for op in ops:
        elems = op["L"] * op["R"]
        dc = 130.0 + 1.05 * elems
        gc = 160.0 + 2.0 * elems
        if dve_t + dc <= gps_t + gc:
            eng_of.append(0)
            dve_t += dc
        else:
            eng_of.append(1)
            gps_t += gc

    with tile.TileContext(nc) as tc:
        with (
            tc.tile_pool(name="const", bufs=1) as cpool,
            tc.tile_pool(name="ework", bufs=4) as epool,
            tc.tile_pool(name="dwork", bufs=4) as dpool,
            tc.tile_pool(name="ps_lg", bufs=4, space="PSUM") as ps_lg,
        ):
            xTa_sb = cpool.tile([DA, BLOC], f32, tag="xTa")
            # chunked load, small first chunk, so sub-block 0 starts early
            xbounds = [0, 128, 512] + [512 * j for j in range(2, BLOC // 512 + 1)]
            for lo, hi in zip(xbounds, xbounds[1:]):
                nc.sync.dma_start(
                    out=xTa_sb[:, lo:hi], in_=xTa_d[:, lo:hi]
                )
            Wa_sb = cpool.tile([DA, SLOTS], f32, tag="Wa")
            nc.sync.dma_start(out=Wa_sb[:], in_=Wa_d[:])

            # persistent output tiles, zeroed once up front; each memset is
            # split into halves across DVE and GPSIMD (~2.8 us each) so all
            # buffers are ready before the first scatter needs them
            osb = []
            HW_ = OUTW // 2
            for j in range(NOSB):
                ot = cpool.tile([P, OUTW], bf16, tag=f"osb{j}")
                nc.vector.memset(ot[:, :HW_], 0.0)
                nc.gpsimd.memset(ot[:, HW_:], 0.0)
                osb.append(ot)

            for i in range(NSUB):
                lg = ps_lg.tile([P, SLOTS], f32, tag="lg")
                nc.tensor.matmul(
                    lg[:, :],
                    xTa_sb[:, i * P:(i + 1) * P],
                    Wa_sb[:, :],
                    start=True,
                    stop=True,
                )
                e = epool.tile([P, SLOTS], f32, tag="e")
                nc.scalar.activation(e[:, :], lg[:, :], AF.Exp)
                den = dpool.tile([P, NZ], f32, tag="den")
                nc.vector.tensor_reduce(
                    den[:, :],
                    e[:, :].rearrange("p (n k) -> p n k", k=KADJ),
                    mybir.AxisListType.X,
                    OP.add,
                )
                rc = dpool.tile([P, NZ], f32, tag="rc")
                nc.vector.reciprocal(rc[:, :], den[:, :])

                ot = osb[i % NOSB]
                et, rt, ott = e[:, :], rc[:, :], ot[:, :]
                for op, which in zip(ops, eng_of):
                    o, c, z0, L, R, s = (
                        op["o"], op["c"], op["z0"], op["L"], op["R"], op["s"],
                    )
                    src = AP(
                        tensor=et.tensor,
                        offset=KADJ * z0 + c,
                        ap=[[SLOTS, P], [KADJ * s, R], [KADJ, L]],
                    )
                    dst = AP(
                        tensor=ott.tensor,
                        offset=(NZ + 1) * z0 + o,
                        ap=[[OUTW, P], [(NZ + 1) * s, R], [NZ + 1, L]],
                    )
                    rca = AP(
                        tensor=rt.tensor,
                        offset=z0,
                        ap=[[NZ, P], [s, R], [1, L]],
                    )
                    eng = nc.vector if which == 0 else nc.gpsimd
                    eng.tensor_tensor(out=dst, in0=src, in1=rca, op=OP.mult)
                nc.sync.dma_start(
                    out=out_d[i * P:(i + 1) * P, :], in_=ot[:, :]
                )
    nc.compile()
    return nc


# --------------------------------------------------------------------------
# Fallback path (general scatter-add): dense scatter-matmul, f32 output
# --------------------------------------------------------------------------

BF = 512
ZPG = 6
NGRP = 14
GRP_NZ = [6] * 13 + [3]
GRP_COL = [486 * g for g in range(14)]
PW_PAIR = [128, 128, 128, 64]
PADW = 448


def _slot_mm(n, k):
    g = n // ZPG
    zz = n % ZPG
    hc = g // 2
    p = hc // 2
    row_hi = 32 * (g % 2) + KADJ * zz + k
    row_pair = 64 * (hc % 2) + row_hi
    return p, row_pair, hc, row_hi


def _build_consts_mm(W, b, idx, mask):
    import ml_dtypes

    bf = ml_dtypes.bfloat16
    W = np.asarray(W, np.float32)
    b = np.asarray(b, np.float32)
    idx = np.asarray(idx)
    mask = np.asarray(mask, np.float32)

    Wa = np.zeros((DA, PADW), np.float32)
    E = np.zeros((NZ, PADW), bf)
    ob = [np.zeros((PW_PAIR[p], NZ), np.float32) for p in range(4)]
    S = np.zeros((P, NZ * NZ), bf)

    for n in range(NZ):
        for k in range(KADJ):
            p, rp, hc, rh = _slot_mm(n, k)
            col = 128 * p + rp
            if mask[n, k] > 0:
                Wa[:D, col] = W[n, :, k]
                Wa[D, col] = b[n, k]
            else:
                Wa[D, col] = NEG
            E[n, col] = 1.0
            ob[p][rp, n] = 1.0
            ocol = n * NZ + int(idx[n, k])
            S[rh, ocol] = 1.0
            S[64 + rh, ocol] = 1.0
    return Wa, E, ob, S


def _build_program_mm(bloc):
    from concourse import bacc, mybir
    import concourse.tile as tile

    f32 = mybir.dt.float32
    bf16 = mybir.dt.bfloat16
    AF = mybir.ActivationFunctionType
    OP = mybir.AluOpType
    nc = bacc.Bacc("TRN2", target_bir_lowering=False, debug=False)

    xTa_d = nc.declare_dram_parameter("xTa", [DA, bloc], f32, isOutput=False)
    Wa_d = nc.declare_dram_parameter("Wa", [DA, PADW], f32, isOutput=False)
    E_d = nc.declare_dram_parameter("E", [NZ, PADW], bf16, isOutput=False)
    ob_d = [
        nc.declare_dram_parameter(f"ob{p}", [PW_PAIR[p], NZ], f32, isOutput=False)
        for p in range(4)
    ]
    S_d = nc.declare_dram_parameter("S", [P, NZ * NZ], bf16, isOutput=False)
    out_d = nc.declare_dram_parameter("out", [bloc, NZ * NZ], f32, isOutput=True)

    n_blk = bloc // BF
    n_sub = BF // P

    with tile.TileContext(nc) as tc:
        with (
            tc.tile_pool(name="const", bufs=1) as cpool,
            tc.tile_pool(name="work", bufs=2) as wpool,
            tc.tile_pool(name="outp", bufs=4) as opool,
            tc.tile_pool(name="ps_log", bufs=2, space="PSUM") as ps_log,
            tc.tile_pool(name="ps_den", bufs=1, space="PSUM") as ps_den,
            tc.tile_pool(name="ps_rf", bufs=2, space="PSUM") as ps_rf,
            tc.tile_pool(name="ps_sc", bufs=3, space="PSUM") as ps_sc,
        ):
            Wa_sb = cpool.tile([DA, PADW], f32, tag="Wa")
            nc.sync.dma_start(out=Wa_sb[:], in_=Wa_d[:])
            E_sb = cpool.tile([NZ, PADW], bf16, tag="E")
            nc.sync.dma_start(out=E_sb[:], in_=E_d[:])
            S_sb = cpool.tile([P, NZ * NZ], bf16, tag="S")
            nc.sync.dma_start(out=S_sb[:], in_=S_d[:])
            ob_sb = []
            for p in range(4):
                t = cpool.tile([PW_PAIR[p], NZ], f32, tag=f"ob{p}")
                nc.sync.dma_start(out=t[:], in_=ob_d[p][:])
                ob_sb.append(t)
            xTa_sb = cpool.tile([DA, bloc], f32, tag="xTa")
            nc.sync.dma_start(out=xTa_sb[:], in_=xTa_d[:])

            def emit_scatter(bs, pcat):
                for i in range(n_sub):
                    osb = opool.tile([P, NZ * NZ], f32, tag="osb")
                    for g in range(NGRP):
                        ncols = GRP_NZ[g] * NZ
                        colg = GRP_COL[g]
                        sc = ps_sc.tile([P, BF], f32, tag="scps")
                        nc.tensor.matmul(
                            sc[:, :ncols],
                            pcat[g // 2][:, i * P:(i + 1) * P],
                            S_sb[:, colg:colg + ncols],
                            start=True,
                            stop=True,
                        )
                        dst = osb[:, colg:colg + ncols]
                        if g % 5 < 3:
                            nc.scalar.copy(dst, sc[:, :ncols])
                        else:
                            nc.vector.tensor_copy(dst, sc[:, :ncols])
                    nc.sync.dma_start(
                        out=out_d[bs + i * P: bs + (i + 1) * P, :], in_=osb[:]
                    )

            prev = None
            for blk in range(n_blk):
                bs = blk * BF
                exT = []
                for p in range(4):
                    pw = PW_PAIR[p]
                    lg = ps_log.tile([P, BF], f32, tag="lg")
                    nc.tensor.matmul(
                        lg[:pw, :],
                        Wa_sb[:, 128 * p:128 * p + pw],
                        xTa_sb[:, bs:bs + BF],
                        start=True,
                        stop=True,
                    )
                    ex = wpool.tile([P, BF], f32, tag=f"exp{p}")
                    nc.scalar.activation(ex[:pw, :], lg[:pw, :], AF.Exp)
                    exT.append(ex)
                den_ps = ps_den.tile([NZ, BF], f32, tag="den")
                for p in range(4):
                    nc.tensor.matmul(
                        den_ps[:, :], ob_sb[p][:], exT[p][:PW_PAIR[p], :],
                        start=(p == 0), stop=(p == 3),
                    )
                rc = wpool.tile([NZ, BF], f32, tag="recipC")
                nc.vector.reciprocal(rc[:], den_ps[:])
                rhi = wpool.tile([NZ, BF], bf16, tag="rhi")
                nc.scalar.copy(rhi[:], rc[:])
                rlo = wpool.tile([NZ, BF], bf16, tag="rlo")
                nc.vector.tensor_tensor(out=rlo[:], in0=rc[:], in1=rhi[:], op=OP.subtract)
                pcat = []
                for p in range(4):
                    pw = PW_PAIR[p]
                    rf = ps_rf.tile([P, BF], f32, tag="rf")
                    nc.tensor.matmul(
                        rf[:pw, :], E_sb[:, 128 * p:128 * p + pw], rhi[:],
                        start=True, stop=False,
                    )
                    nc.tensor.matmul(
                        rf[:pw, :], E_sb[:, 128 * p:128 * p + pw], rlo[:],
                        start=False, stop=True,
                    )
                    for h in range(2 if pw == 128 else 1):
                        sl = slice(64 * h, 64 * h + 64)
                        pt = wpool.tile([64, BF], f32, tag=f"pt{2 * p + h}")
                        nc.vector.tensor_tensor(
                            out=pt[:, :], in0=exT[p][sl, :], in1=rf[sl, :], op=OP.mult
                        )
                        pc = wpool.tile([P, BF], bf16, tag=f"pcat{2 * p + h}")
                        nc.scalar.copy(pc[:64, :], pt[:, :])
                        nc.vector.tensor_tensor(
                            out=pc[64:, :],
                            in0=pt[:, :],
                            in1=pc[:64, :],
                            op=OP.subtract,
                        )
                        pcat.append(pc)
                if prev is not None:
                    emit_scatter(*prev)
                prev = (bs, pcat)
            emit_scatter(*prev)
    nc.compile()
    return nc


# --------------------------------------------------------------------------
# Entry
# --------------------------------------------------------------------------

def _install_ntff_hook():
    """Shim antenv.axon_hooks (absent in this image) so trace=True can drive
    NRT profiling through libaxon_pjrt.so. Only used for self-profiling."""
    import types

    try:
        import antenv

        try:
            from antenv.axon_hooks import get_axon_ntff_profile_hook  # noqa: F401

            return True
        except ImportError:
            pass
        if "/root/.axon_site" not in sys.path:
            sys.path.insert(0, "/root/.axon_site")
        from trn_agent_boot.trn_boot import _ntff_profile_via_ctypes

        hook = _ntff_profile_via_ctypes("/opt/axon/libaxon_pjrt.so")
        mod = types.ModuleType("antenv.axon_hooks")
        state = {"hook": hook}
        mod.get_axon_ntff_profile_hook = lambda: state["hook"]
        mod.set_axon_ntff_profile_hook = lambda h: state.update(hook=h)
        sys.modules["antenv.axon_hooks"] = mod
        antenv.axon_hooks = mod
        return hook is not None
    except Exception as e:  # profiling is best-effort; never break the run
        print("ntff hook install failed:", e)
        return False


def _make_xta_maps(obs, consts):
    in_maps = []
    for i in range(NCORES):
        shard = obs[i * BLOC:(i + 1) * BLOC, :D]
        xTa = np.concatenate(
            [np.ascontiguousarray(shard.T), np.ones((1, BLOC), np.float32)], axis=0
        )
        m = dict(consts)
        m["xTa"] = np.ascontiguousarray(xTa)
        in_maps.append(m)
    return in_maps


def kernel(obs, W, b, idx, mask):
    from concourse.bass_utils import run_bass_kernel_spmd

    global LAST_RESULTS
    trace = bool(int(os.environ.get("KBT_TRACE", "0")))
    if trace:
        trace = _install_ntff_hook()
    obs = np.asarray(obs, np.float32)
    idx = np.asarray(idx)
    mask = np.asarray(mask, np.float32)

    plan = _plan_scatter(idx, mask)
    if plan is not None:
        assign, ops = plan
        Wa = _build_wa(W, b, assign)
        nc = _build_program_fast(ops)
        in_maps = _make_xta_maps(obs, {"Wa": Wa})
        br = run_bass_kernel_spmd(nc, in_maps, list(range(NCORES)), trace=trace)
        LAST_RESULTS = br
        out = np.concatenate(
            [
                np.asarray(br.results[i]["out"]).astype(np.float32)
                for i in range(NCORES)
            ],
            axis=0,
        )
        return out.reshape(BATCH, NZ, NZ)

    # general scatter-add fallback
    Wa, E, ob, S = _build_consts_mm(W, b, idx, mask)
    nc = _build_program_mm(BLOC)
    consts = {"Wa": Wa, "E": E, "S": S}
    for p in range(4):
        consts[f"ob{p}"] = ob[p]
    in_maps = _make_xta_maps(obs, consts)
    br = run_bass_kernel_spmd(nc, in_maps, list(range(NCORES)), trace=trace)
    LAST_RESULTS = br
    out = np.concatenate([br.results[i]["out"] for i in range(NCORES)], axis=0)
    return out.reshape(BATCH, NZ, NZ)
